# revision 1
# baseline (speedup 1.0000x reference)
"""Trainium2 Bass kernel for nn_Node_Convolution (GNN message passing).

Reference computation:
    z   = concat([x[src], x[tgt], edge_attr], -1)           # [E, 384]
    msg = sigmoid(z @ Wf + bf) * softplus(z @ Ws + bs)      # [E, 128]
    agg = segment_sum(msg, src, N)                          # [N, 128]
    out = softplus(x + batchnorm(agg))                      # [N, 128]

Strategy (edge-parallel across 8 NeuronCores):
  * Host: sort edges by source node, split nodes into 8 contiguous ranges with
    ~equal edge counts.  Each core gets its edge shard; x / weights replicated.
  * Host packs each core's (sorted) edges into 128-edge tiles such that no
    node's edge-run straddles a tile and each tile touches <= 32 distinct
    source nodes.  Edge features are laid out feature-major per tile so the
    device streams everything with plain contiguous DMA.
  * Device kernel K1 (per core, SPMD): for each 128-edge tile
      - 3 accumulating bf16 matmuls  psum[e, 0:256] = z^T_tile.T @ [ -Wf | Ws ]
        (Wf negated on host so the sigmoid can use Exp(-f) directly)
      - ACT: e = Exp(psum) ; sp = Ln(e_s + 1)   (softplus, one act table)
      - DVE: sig = 1 / (1 + e_f) ; msg = sig * sp
      - one-hot "selection" matmul S.T @ msg sums duplicate source rows into
        <= 32 unique slot rows (exact fp32 dedup in PSUM)
      - slot rows written contiguously to a staging output; per-feature
        sum / sum-of-squares accumulated via ones-matmuls (BN statistics).
  * Host: staging rows -> per-node agg is a pure permutation (no node spans
    two tiles); the 8 partial stat vectors are reduced to BN scale/shift
    (128 floats each).
  * Device kernel K2 (per core): out = softplus(x + agg * a + b), streamed.

All FLOPs (matmuls, activations, segment-sum dedup, BN application) run on
device.  The host does only data layout: sort/permute/shard/transpose and the
[8 x 128 x 2] -> [128] statistics reduction.  (The SWDGE ucode gather/scatter
instructions are non-functional under this execution stack, so edge gathers
are materialized host-side during sharding.)
"""
import sys
sys.path.insert(0, "/opt/trn_rl_repo")

import numpy as np
import ml_dtypes

from concourse import bass, mybir
import concourse.bacc as bacc
import concourse.tile as tile
from concourse.bass_utils import run_bass_kernel_spmd

F32 = mybir.dt.float32
BF16 = mybir.dt.bfloat16

N_CORES = 8
D = 128
M_SLOTS = 32          # max distinct source nodes per 128-edge tile
T_C = 32              # tiles per DMA chunk
GROUP = 2             # tiles per staging write group (2 * 32 slots = 64 rows)
BN_EPS = 1e-5

_BUILD_CACHE = {}


# --------------------------------------------------------------------------
# host-side packing
# --------------------------------------------------------------------------

def _partition_nodes(src_sorted, n_nodes, n_edges):
    """Split nodes into N_CORES contiguous ranges with ~equal edge counts."""
    deg = np.bincount(src_sorted, minlength=n_nodes)
    cum = np.cumsum(deg)
    bounds = [0]
    for k in range(1, N_CORES):
        bounds.append(int(np.searchsorted(cum, k * n_edges / N_CORES)))
    bounds.append(n_nodes)
    return [int(b) for b in bounds], deg


def _pack_core(nodes, degs):
    """Pack node edge-runs into tiles of <=128 edges, <=M_SLOTS distinct nodes,
    never splitting a run (unless a single run exceeds 128 edges).
    Returns list of tiles; each tile is a list of (node, run_len)."""
    tiles = []
    cur = []
    cur_edges = 0
    for node, L in zip(nodes, degs):
        L = int(L)
        while L > 128:  # pathological high-degree node: split (host will sum)
            if cur:
                tiles.append(cur)
                cur, cur_edges = [], 0
            tiles.append([(node, 128)])
            L -= 128
        if L == 0:
            continue
        if cur_edges + L > 128 or len(cur) >= M_SLOTS:
            tiles.append(cur)
            cur, cur_edges = [], 0
        cur.append((node, L))
        cur_edges += L
    if cur:
        tiles.append(cur)
    return tiles


def _prepare(x, edge_attr, edge_source, edge_target, Wf, bf, Ws, bs):
    n_nodes, d = x.shape
    n_edges = edge_source.shape[0]
    src = np.asarray(edge_source).astype(np.int64)
    tgt = np.asarray(edge_target).astype(np.int64)

    order = np.argsort(src, kind="stable")
    src_s = src[order]
    bounds, deg = _partition_nodes(src_s, n_nodes, n_edges)

    cores = []
    for c in range(N_CORES):
        lo, hi = bounds[c], bounds[c + 1]
        nodes = np.arange(lo, hi)[deg[lo:hi] > 0]
        degs = deg[nodes]
        tiles = _pack_core(nodes, degs)
        cores.append((lo, hi, tiles))

    n_tiles = max(len(t[2]) for t in cores)
    n_tiles = -(-n_tiles // T_C) * T_C                 # round up to chunk multiple
    e_pad = n_tiles * 128
    n_pad = max(hi - lo for lo, hi, _ in cores)
    n_pad = -(-n_pad // 128) * 128

    # per-core edge start offsets in sorted order
    cum = np.concatenate([[0], np.cumsum(deg)])

    per_core = []
    for c in range(N_CORES):
        lo, hi, tiles = cores[c]
        perm = np.zeros(e_pad, dtype=np.int64)          # original edge index per slot
        winrel = np.full((128, n_tiles), -1.0, dtype=np.float32)
        stage_rows = []                                  # (staging_row, node_local)
        consumed = {}
        for t, tl in enumerate(tiles):
            e_off = 0
            for j, (node, L) in enumerate(tl):
                stage_rows.append((t * M_SLOTS + j, node - lo))
                used = consumed.get(node, 0)
                start = cum[node] + used
                perm[t * 128 + e_off:t * 128 + e_off + L] = order[start:start + L]
                winrel[e_off:e_off + L, t] = j
                consumed[node] = used + L
                e_off += L
        per_core.append(dict(lo=lo, hi=hi, tiles=tiles, perm=perm, winrel=winrel,
                             stage_rows=stage_rows))

    x32 = np.asarray(x, dtype=np.float32)
    ea32 = np.asarray(edge_attr, dtype=np.float32)
    Wf32 = np.asarray(Wf, dtype=np.float32)
    Ws32 = np.asarray(Ws, dtype=np.float32)
    bf32 = np.asarray(bf, dtype=np.float32)
    bs32 = np.asarray(bs, dtype=np.float32)
    with_bias = bool(np.any(bf32) or np.any(bs32))

    wsrc = np.concatenate([-Wf32[0:128], Ws32[0:128]], axis=1).astype(ml_dtypes.bfloat16)
    wtgt = np.concatenate([-Wf32[128:256], Ws32[128:256]], axis=1).astype(ml_dtypes.bfloat16)
    wea = np.concatenate([-Wf32[256:384], Ws32[256:384]], axis=1).astype(ml_dtypes.bfloat16)
    bias_row = np.concatenate([-bf32, bs32]).reshape(1, 256).astype(ml_dtypes.bfloat16)

    in_maps = []
    for c in range(N_CORES):
        pc = per_core[c]
        perm = pc["perm"]
        srcT = np.ascontiguousarray(x32[src[perm]].T.astype(ml_dtypes.bfloat16))
        tgtT = np.ascontiguousarray(x32[tgt[perm]].T.astype(ml_dtypes.bfloat16))
        eaT = np.ascontiguousarray(ea32[perm].T.astype(ml_dtypes.bfloat16))
        m = dict(eaT=eaT, srcT=srcT, tgtT=tgtT, winrel=pc["winrel"],
                 wsrc=wsrc, wtgt=wtgt, wea=wea)
        if with_bias:
            m["bias_row"] = bias_row
        in_maps.append(m)

    meta = dict(n_tiles=n_tiles, n_pad=n_pad, per_core=per_core,
                with_bias=with_bias, n_nodes=n_nodes)
    return in_maps, meta


# --------------------------------------------------------------------------
# device kernels
# --------------------------------------------------------------------------

def _build_k1(n_tiles, with_bias):
    key = ("k1", n_tiles, with_bias)
    if key in _BUILD_CACHE:
        return _BUILD_CACHE[key]
    e_pad = n_tiles * 128
    n_groups = n_tiles // GROUP
    n_chunks = n_tiles // T_C
    nc = bacc.Bacc(None, debug=False, num_devices=N_CORES)

    eaT = nc.declare_dram_parameter("eaT", [128, e_pad], BF16, isOutput=False)
    srcT = nc.declare_dram_parameter("srcT", [128, e_pad], BF16, isOutput=False)
    tgtT = nc.declare_dram_parameter("tgtT", [128, e_pad], BF16, isOutput=False)
    winrel = nc.declare_dram_parameter("winrel", [128, n_tiles], F32, isOutput=False)
    wsrc = nc.declare_dram_parameter("wsrc", [128, 256], BF16, isOutput=False)
    wtgt = nc.declare_dram_parameter("wtgt", [128, 256], BF16, isOutput=False)
    wea = nc.declare_dram_parameter("wea", [128, 256], BF16, isOutput=False)
    if with_bias:
        bias_row = nc.declare_dram_parameter("bias_row", [1, 256], BF16, isOutput=False)
    # stats [128,2] ride in rows [n_tiles*M_SLOTS, +128) cols 0:2 of staging --
    # a second ExternalOutput costs ~80ms of per-dispatch overhead on this stack.
    staging = nc.declare_dram_parameter("staging", [n_tiles * M_SLOTS + 128, 128], F32, isOutput=True)

    with tile.TileContext(nc) as tc:
        with (
            tc.tile_pool(name="res", bufs=1) as res,
            tc.tile_pool(name="chunk", bufs=2) as cpool,
            tc.tile_pool(name="work", bufs=3) as wpool,
            tc.tile_pool(name="pse", bufs=2, space="PSUM") as pse_pool,
            tc.tile_pool(name="psl", bufs=2, space="PSUM") as psl_pool,
            tc.tile_pool(name="pst", bufs=1, space="PSUM") as pst_pool,
        ):
            # resident constants
            w1 = res.tile([128, 256], BF16, tag="w1")
            w2 = res.tile([128, 256], BF16, tag="w2")
            w3 = res.tile([128, 256], BF16, tag="w3")
            nc.sync.dma_start(w1[:], wsrc[:, :])
            nc.sync.dma_start(w2[:], wtgt[:, :])
            nc.sync.dma_start(w3[:], wea[:, :])
            wr_sb = res.tile([128, n_tiles], F32, tag="wr")
            nc.sync.dma_start(wr_sb[:], winrel[:, :])
            iota_i = res.tile([128, M_SLOTS], mybir.dt.int32, tag="ii")
            nc.gpsimd.iota(iota_i[:], pattern=[[1, M_SLOTS]], base=0, channel_multiplier=0)
            iota_f = res.tile([128, M_SLOTS], F32, tag="if")
            nc.vector.tensor_copy(iota_f[:], iota_i[:])
            ones_f = res.tile([128, 1], F32, tag="o1")
            nc.vector.memset(ones_f[:], 1.0)
            if with_bias:
                ones_bf = res.tile([1, 128], BF16, tag="ob")
                nc.vector.memset(ones_bf[:], 1.0)
                brow = res.tile([1, 256], BF16, tag="br")
                nc.sync.dma_start(brow[:], bias_row[:, :])

            sum_ps = pst_pool.tile([128, 1], F32, tag="sum")
            sq_ps = pst_pool.tile([128, 1], F32, tag="sq")

            for ch in range(n_chunks):
                c0 = ch * T_C * 128
                ea_c = cpool.tile([128, T_C * 128], BF16, tag="ea")
                src_c = cpool.tile([128, T_C * 128], BF16, tag="src")
                tgt_c = cpool.tile([128, T_C * 128], BF16, tag="tgt")
                nc.sync.dma_start(ea_c[:], eaT[:, c0:c0 + T_C * 128])
                nc.sync.dma_start(src_c[:], srcT[:, c0:c0 + T_C * 128])
                nc.sync.dma_start(tgt_c[:], tgtT[:, c0:c0 + T_C * 128])
                for tt in range(T_C):
                    t = ch * T_C + tt
                    g, j = divmod(t, GROUP)
                    sl = slice(tt * 128, (tt + 1) * 128)
                    ps_e = pse_pool.tile([128, 256], F32, tag="pse")
                    nc.tensor.matmul(ps_e[:], lhsT=src_c[:, sl], rhs=w1[:],
                                     start=True, stop=False)
                    nc.tensor.matmul(ps_e[:], lhsT=tgt_c[:, sl], rhs=w2[:],
                                     start=False, stop=False)
                    nc.tensor.matmul(ps_e[:], lhsT=ea_c[:, sl], rhs=w3[:],
                                     start=False, stop=not with_bias)
                    if with_bias:
                        nc.tensor.matmul(ps_e[:], lhsT=ones_bf[:], rhs=brow[:],
                                         start=False, stop=True)
                    # e = Exp([-f | s]);  sp = Ln(e_s + 1);  sig = 1/(1+e_f)
                    e_all = wpool.tile([128, 256], F32, tag="e")
                    nc.scalar.activation(e_all[:], ps_e[:], mybir.ActivationFunctionType.Exp)
                    sp = wpool.tile([128, 128], F32, tag="sp")
                    nc.scalar.activation(sp[:], e_all[:, 128:256],
                                         mybir.ActivationFunctionType.Ln, bias=1.0)
                    den = wpool.tile([128, 128], F32, tag="den")
                    nc.vector.tensor_scalar_add(den[:], e_all[:, 0:128], 1.0)
                    sig = wpool.tile([128, 128], F32, tag="sig")
                    nc.vector.reciprocal(sig[:], den[:])
                    msg = wpool.tile([128, 128], BF16, tag="msg")
                    nc.vector.tensor_mul(msg[:], sig[:], sp[:])
                    # selection matrix: S[e, j] = (winrel[e, t] == j)
                    S = wpool.tile([128, M_SLOTS], BF16, tag="S")
                    nc.vector.tensor_tensor(
                        out=S[:],
                        in0=wr_sb[:, t:t + 1].to_broadcast([128, M_SLOTS]),
                        in1=iota_f[:],
                        op=mybir.AluOpType.is_equal,
                    )
                    if j == 0:
                        ps_slot = psl_pool.tile([64, 128], F32, tag="psl")
                        _BUILD_CACHE["_cur_slot"] = ps_slot
                    else:
                        ps_slot = _BUILD_CACHE["_cur_slot"]
                    nc.tensor.matmul(ps_slot[j * M_SLOTS:(j + 1) * M_SLOTS, :],
                                     lhsT=S[:], rhs=msg[:], start=True, stop=True)
                    if j == GROUP - 1:
                        scat = wpool.tile([64, 128], F32, tag="scat")
                        nc.vector.tensor_copy(scat[:], ps_slot[:])
                        nc.sync.dma_start(staging[g * 64:(g + 1) * 64, :], scat[:])
                        sq = wpool.tile([64, 128], F32, tag="sqt")
                        nc.vector.tensor_mul(sq[:], scat[:], scat[:])
                        nc.tensor.matmul(sum_ps[:], lhsT=scat[:], rhs=ones_f[0:64, :],
                                         start=(g == 0), stop=(g == n_groups - 1),
                                         skip_group_check=True)
                        nc.tensor.matmul(sq_ps[:], lhsT=sq[:], rhs=ones_f[0:64, :],
                                         start=(g == 0), stop=(g == n_groups - 1),
                                         skip_group_check=True)
            st_sb = res.tile([128, 2], F32, tag="st")
            nc.vector.tensor_copy(st_sb[:, 0:1], sum_ps[:])
            nc.vector.tensor_copy(st_sb[:, 1:2], sq_ps[:])
            nc.sync.dma_start(staging[n_tiles * M_SLOTS:n_tiles * M_SLOTS + 128, 0:2], st_sb[:])

    _BUILD_CACHE.pop("_cur_slot", None)
    nc.compile()
    _BUILD_CACHE[key] = nc
    return nc


def _build_k2(n_pad):
    key = ("k2", n_pad)
    if key in _BUILD_CACHE:
        return _BUILD_CACHE[key]
    nt = n_pad // 128
    nc = bacc.Bacc(None, debug=False, num_devices=N_CORES)
    agg = nc.declare_dram_parameter("agg", [n_pad, 128], F32, isOutput=False)
    x_sl = nc.declare_dram_parameter("x_sl", [n_pad, 128], F32, isOutput=False)
    a_rep = nc.declare_dram_parameter("a_rep", [128, 128], F32, isOutput=False)
    b_rep = nc.declare_dram_parameter("b_rep", [128, 128], F32, isOutput=False)
    out = nc.declare_dram_parameter("out", [n_pad, 128], F32, isOutput=True)

    with tile.TileContext(nc) as tc:
        with (
            tc.tile_pool(name="res", bufs=1) as res,
            tc.tile_pool(name="work", bufs=4) as wpool,
        ):
            a_sb = res.tile([128, 128], F32, tag="a")
            b_sb = res.tile([128, 128], F32, tag="b")
            nc.sync.dma_start(a_sb[:], a_rep[:, :])
            nc.sync.dma_start(b_sb[:], b_rep[:, :])
            for t in range(nt):
                rs = slice(t * 128, (t + 1) * 128)
                ag = wpool.tile([128, 128], F32, tag="ag")
                xt = wpool.tile([128, 128], F32, tag="xt")
                nc.sync.dma_start(ag[:], agg[rs, :])
                nc.sync.dma_start(xt[:], x_sl[rs, :])
                pre = wpool.tile([128, 128], F32, tag="pre")
                nc.vector.tensor_mul(pre[:], ag[:], a_sb[:])
                nc.vector.tensor_add(pre[:], pre[:], b_sb[:])
                nc.vector.tensor_add(pre[:], pre[:], xt[:])
                ex = wpool.tile([128, 128], F32, tag="ex")
                nc.scalar.activation(ex[:], pre[:], mybir.ActivationFunctionType.Exp)
                ot = wpool.tile([128, 128], F32, tag="ot")
                nc.scalar.activation(ot[:], ex[:], mybir.ActivationFunctionType.Ln, bias=1.0)
                nc.sync.dma_start(out[rs, :], ot[:])
    nc.compile()
    _BUILD_CACHE[key] = nc
    return nc


# --------------------------------------------------------------------------
# entry point
# --------------------------------------------------------------------------

def kernel(x, edge_attr, edge_source, edge_target, Wf, bf, Ws, bs, gamma, beta):
    x = np.asarray(x)
    n_nodes = x.shape[0]
    in_maps, meta = _prepare(x, edge_attr, edge_source, edge_target, Wf, bf, Ws, bs)
    n_tiles, n_pad = meta["n_tiles"], meta["n_pad"]

    nc1 = _build_k1(n_tiles, meta["with_bias"])
    res1 = run_bass_kernel_spmd(nc1, in_maps, core_ids=list(range(N_CORES)))

    # host: reduce stats, permute staging rows into per-node agg slices
    x32 = np.asarray(x, dtype=np.float32)
    tot = np.zeros((128, 2), dtype=np.float64)
    aggs = []
    for c in range(N_CORES):
        r = res1.results[c]
        tot += np.asarray(r["staging"][n_tiles * M_SLOTS:, 0:2], dtype=np.float64)
        pc = meta["per_core"][c]
        lo, hi = pc["lo"], pc["hi"]
        agg = np.zeros((n_pad, 128), dtype=np.float32)
        if pc["stage_rows"]:
            rows = np.array([sr for sr, _ in pc["stage_rows"]])
            nloc = np.array([nl for _, nl in pc["stage_rows"]])
            np.add.at(agg, nloc, np.asarray(r["staging"])[rows])
        aggs.append(agg)

    mean = tot[:, 0] / n_nodes
    var = tot[:, 1] / n_nodes - mean * mean
    a = np.asarray(gamma, dtype=np.float64) / np.sqrt(var + BN_EPS)
    b = np.asarray(beta, dtype=np.float64) - mean * a
    a_rep = np.tile(a.astype(np.float32), (128, 1))
    b_rep = np.tile(b.astype(np.float32), (128, 1))

    in_maps2 = []
    for c in range(N_CORES):
        pc = meta["per_core"][c]
        lo, hi = pc["lo"], pc["hi"]
        x_sl = np.zeros((n_pad, 128), dtype=np.float32)
        x_sl[0:hi - lo] = x32[lo:hi]
        in_maps2.append(dict(agg=aggs[c], x_sl=x_sl, a_rep=a_rep, b_rep=b_rep))

    nc2 = _build_k2(n_pad)
    res2 = run_bass_kernel_spmd(nc2, in_maps2, core_ids=list(range(N_CORES)))

    out = np.empty((n_nodes, 128), dtype=np.float32)
    for c in range(N_CORES):
        pc = meta["per_core"][c]
        lo, hi = pc["lo"], pc["hi"]
        out[lo:hi] = np.asarray(res2.results[c]["out"])[0:hi - lo]
    return out



# revision 9
# speedup vs baseline: 1.4889x; 1.4889x over previous
"""Trainium2 Bass kernel for nn_Node_Convolution (GNN message passing).

Reference computation:
    z   = concat([x[src], x[tgt], edge_attr], -1)           # [E, 384]
    msg = sigmoid(z @ Wf + bf) * softplus(z @ Ws + bs)      # [E, 128]
    agg = segment_sum(msg, src, N)                          # [N, 128]
    out = softplus(x + batchnorm(agg))                      # [N, 128]

Strategy — ONE fused SPMD kernel on 8 NeuronCores (the per-dispatch
overhead of this execution stack is ~85 ms, so a single dispatch wins):
  * Host: sort edges by source node, split nodes into 8 contiguous ranges
    with ~equal edge counts.  Each core's range is cut into 128-node
    windows; each window's edges are packed into a FIXED number (TPW) of
    128-edge tiles (padded), so the instruction stream is identical on
    every core (SPMD) while tile contents differ.
  * Everything on device is feature-major ([feat, node/edge]) so the BN
    statistics live in the partition dim as per-partition scalars:
      - per tile: psum[e, 0:256] = z_tile @ [Wf | Ws] via 3 accumulating
        bf16 matmuls; ACT Sigmoid / ACT Softplus; DVE mul -> msg [e, f]
      - scatter: one-hot S[e, n] = (winrel[e] == n); matmul
        psw[f, n] += msg.T @ S accumulates the window's segment-sum in
        PSUM across its TPW tiles (exact fp32 dedup)
      - window epilogue: ACT copy psw -> agg slice with accum_out giving
        the per-feature sum; DVE tensor_tensor_reduce gives sum-of-squares
  * BN stats ([128, 2] per core) are AllReduce'd across the 8 cores
    INSIDE the kernel (gpsimd collective_compute via DRAM bounce), then
    a = gamma*rsqrt(var+eps), b = beta - mean*a as [128,1] columns.
  * Phase 2 (same dispatch): out[f, n] = ACT Softplus(x + ACT(agg*a + b))
    streamed per window; x is preloaded feature-major.
  * Host: de-transpose the 8 output shards into the full [N, 128] output.

The host does only data layout (sort/shard/gather/transpose); all FLOPs,
the segment-sum, the BN reduction and application run on device.
(SWDGE ucode gather/scatter is non-functional under this stack, so edge
gathers are materialized host-side during sharding.)
"""
import sys
sys.path.insert(0, "/opt/trn_rl_repo")

import numpy as np
import ml_dtypes

from concourse import bass, mybir
import concourse.bacc as bacc
import concourse.tile as tile
from concourse.bass_utils import run_bass_kernel_spmd

F32 = mybir.dt.float32
BF16 = mybir.dt.bfloat16

N_CORES = 8
D = 128
BN_EPS = 1e-5

_BUILD_CACHE = {}


# --------------------------------------------------------------------------
# host-side packing
# --------------------------------------------------------------------------

def _partition_nodes(src_sorted, n_nodes, n_edges):
    """Split nodes into N_CORES contiguous ranges with ~equal edge counts."""
    deg = np.bincount(src_sorted, minlength=n_nodes)
    cum = np.cumsum(deg)
    bounds = [0]
    for k in range(1, N_CORES):
        bounds.append(int(np.searchsorted(cum, k * n_edges / N_CORES)))
    bounds.append(n_nodes)
    return [int(b) for b in bounds], deg


def _prepare(x, edge_attr, edge_source, edge_target, Wf, bf, Ws, bs):
    n_nodes, d = x.shape
    n_edges = edge_source.shape[0]
    src = np.asarray(edge_source).astype(np.int64)
    tgt = np.asarray(edge_target).astype(np.int64)

    order = np.argsort(src, kind="stable")
    src_s = src[order]
    bounds, deg = _partition_nodes(src_s, n_nodes, n_edges)
    cum = np.concatenate([[0], np.cumsum(deg)])

    # uniform kernel structure: nw windows x TPW tiles on every core
    nw = max(-(-(bounds[c + 1] - bounds[c]) // 128) for c in range(N_CORES))
    tpw = 1
    for c in range(N_CORES):
        lo, hi = bounds[c], bounds[c + 1]
        for w in range(-(-(hi - lo) // 128)):
            a = lo + w * 128
            b = min(a + 128, hi)
            tpw = max(tpw, -(-int(cum[b] - cum[a]) // 128))
    n_tiles = nw * tpw
    e_pad = n_tiles * 128
    n_pad = nw * 128

    x32 = np.asarray(x, dtype=np.float32)
    ea32 = np.asarray(edge_attr, dtype=np.float32)
    Wf32 = np.asarray(Wf, dtype=np.float32)
    Ws32 = np.asarray(Ws, dtype=np.float32)
    bf32 = np.asarray(bf, dtype=np.float32)
    bs32 = np.asarray(bs, dtype=np.float32)
    with_bias = bool(np.any(bf32) or np.any(bs32))

    # f-block negated so sigmoid(f) = 1/(1 + Exp(-f)) shares the Exp with
    # softplus (ACT has no Sigmoid/Softplus in the Ln/Exp table)
    wsrc = np.concatenate([-Wf32[0:128], Ws32[0:128]], axis=1).astype(ml_dtypes.bfloat16)
    wtgt = np.concatenate([-Wf32[128:256], Ws32[128:256]], axis=1).astype(ml_dtypes.bfloat16)
    wea = np.concatenate([-Wf32[256:384], Ws32[256:384]], axis=1).astype(ml_dtypes.bfloat16)
    bias_row = np.concatenate([-bf32, bs32]).reshape(1, 256).astype(ml_dtypes.bfloat16)

    in_maps = []
    for c in range(N_CORES):
        lo, hi = bounds[c], bounds[c + 1]
        perm = np.full(e_pad, -1, dtype=np.int64)
        winrel = np.full((128, n_tiles), -1.0, dtype=np.float32)
        for w in range(-(-(hi - lo) // 128)):
            a = lo + w * 128
            b = min(a + 128, hi)
            s0, s1 = int(cum[a]), int(cum[b])
            K = s1 - s0
            if K == 0:
                continue
            base = w * tpw * 128
            perm[base:base + K] = order[s0:s1]
            idx = np.arange(K)
            winrel[idx % 128, w * tpw + idx // 128] = (src_s[s0:s1] - a).astype(np.float32)
        mask = perm >= 0
        pm = perm[mask]

        def _featT(rows):
            a_ = np.zeros((e_pad, D), dtype=np.float32)
            a_[mask] = rows
            return np.ascontiguousarray(a_.T.astype(ml_dtypes.bfloat16))

        srcT = _featT(x32[src[pm]])
        tgtT = _featT(x32[tgt[pm]])
        eaT = _featT(ea32[pm])
        xT = np.zeros((D, n_pad), dtype=np.float32)
        xT[:, 0:hi - lo] = x32[lo:hi].T
        m = dict(eaT=eaT, srcT=srcT, tgtT=tgtT, winrel=winrel,
                 wsrc=wsrc, wtgt=wtgt, wea=wea, xT=xT)
        if with_bias:
            m["bias_row"] = bias_row
        in_maps.append(m)

    meta = dict(nw=nw, tpw=tpw, n_pad=n_pad, bounds=bounds,
                with_bias=with_bias, n_nodes=n_nodes)
    return in_maps, meta


# --------------------------------------------------------------------------
# fused device kernel
# --------------------------------------------------------------------------

def _build_fused(nw, tpw, n_nodes, with_bias):
    key = ("fused", nw, tpw, n_nodes, with_bias)
    if key in _BUILD_CACHE:
        return _BUILD_CACHE[key]
    n_tiles = nw * tpw
    e_pad = n_tiles * 128
    n_pad = nw * 128
    nc = bacc.Bacc(None, debug=False, num_devices=N_CORES)

    eaT = nc.declare_dram_parameter("eaT", [128, e_pad], BF16, isOutput=False)
    srcT = nc.declare_dram_parameter("srcT", [128, e_pad], BF16, isOutput=False)
    tgtT = nc.declare_dram_parameter("tgtT", [128, e_pad], BF16, isOutput=False)
    winrel = nc.declare_dram_parameter("winrel", [128, n_tiles], F32, isOutput=False)
    wsrc = nc.declare_dram_parameter("wsrc", [128, 256], BF16, isOutput=False)
    wtgt = nc.declare_dram_parameter("wtgt", [128, 256], BF16, isOutput=False)
    wea = nc.declare_dram_parameter("wea", [128, 256], BF16, isOutput=False)
    xT = nc.declare_dram_parameter("xT", [128, n_pad], F32, isOutput=False)
    gcol = nc.declare_dram_parameter("gcol", [128, 1], F32, isOutput=False)
    bcol = nc.declare_dram_parameter("bcol", [128, 1], F32, isOutput=False)
    if with_bias:
        bias_row = nc.declare_dram_parameter("bias_row", [1, 256], BF16, isOutput=False)
    outT = nc.declare_dram_parameter("outT", [128, n_pad], F32, isOutput=True)

    AF = mybir.ActivationFunctionType
    with tile.TileContext(nc) as tc:
        with (
            tc.tile_pool(name="res", bufs=1) as res,
            tc.tile_pool(name="chunk", bufs=2) as cpool,
            tc.tile_pool(name="work", bufs=3) as wpool,
            tc.tile_pool(name="pse", bufs=2, space="PSUM") as pse_pool,
            tc.tile_pool(name="psw", bufs=2, space="PSUM") as psw_pool,
            tc.tile_pool(name="dram", bufs=2, space="DRAM") as dram,
        ):
            # resident constants / state
            w1 = res.tile([128, 256], BF16, tag="w1")
            w2 = res.tile([128, 256], BF16, tag="w2")
            w3 = res.tile([128, 256], BF16, tag="w3")
            nc.sync.dma_start(w1[:], wsrc[:, :])
            nc.sync.dma_start(w2[:], wtgt[:, :])
            nc.sync.dma_start(w3[:], wea[:, :])
            wr_sb = res.tile([128, n_tiles], F32, tag="wr")
            nc.sync.dma_start(wr_sb[:], winrel[:, :])
            xsb = res.tile([128, n_pad], F32, tag="x")
            nc.sync.dma_start(xsb[:], xT[:, :])
            g_sb = res.tile([128, 1], F32, tag="g")
            b_sb = res.tile([128, 1], F32, tag="b")
            nc.sync.dma_start(g_sb[:], gcol[:, :])
            nc.sync.dma_start(b_sb[:], bcol[:, :])
            iota_i = res.tile([128, 128], mybir.dt.int32, tag="ii")
            nc.gpsimd.iota(iota_i[:], pattern=[[1, 128]], base=0, channel_multiplier=0)
            iota_f = res.tile([128, 128], F32, tag="if")
            nc.vector.tensor_copy(iota_f[:], iota_i[:])
            if with_bias:
                ones_bf = res.tile([1, 128], BF16, tag="ob")
                nc.vector.memset(ones_bf[:], 1.0)
                brow = res.tile([1, 256], BF16, tag="br")
                nc.sync.dma_start(brow[:], bias_row[:, :])

            agg_sb = res.tile([128, n_pad], F32, tag="agg")
            stats_sum = res.tile([128, nw], F32, tag="ssum")
            stats_sq = res.tile([128, nw], F32, tag="ssq")

            # ---------------- phase 1: edge messages + segment sum ----------
            for w in range(nw):
                c0 = w * tpw * 128
                ea_c = cpool.tile([128, tpw * 128], BF16, tag="ea")
                src_c = cpool.tile([128, tpw * 128], BF16, tag="src")
                tgt_c = cpool.tile([128, tpw * 128], BF16, tag="tgt")
                nc.sync.dma_start(ea_c[:], eaT[:, c0:c0 + tpw * 128])
                nc.sync.dma_start(src_c[:], srcT[:, c0:c0 + tpw * 128])
                nc.sync.dma_start(tgt_c[:], tgtT[:, c0:c0 + tpw * 128])
                psw = psw_pool.tile([128, 128], F32, tag="psw")
                for j in range(tpw):
                    t = w * tpw + j
                    sl = slice(j * 128, (j + 1) * 128)
                    ps_e = pse_pool.tile([128, 256], F32, tag="pse")
                    nc.tensor.matmul(ps_e[:], lhsT=src_c[:, sl], rhs=w1[:],
                                     start=True, stop=False)
                    nc.tensor.matmul(ps_e[:], lhsT=tgt_c[:, sl], rhs=w2[:],
                                     start=False, stop=False)
                    nc.tensor.matmul(ps_e[:], lhsT=ea_c[:, sl], rhs=w3[:],
                                     start=False, stop=not with_bias)
                    if with_bias:
                        nc.tensor.matmul(ps_e[:], lhsT=ones_bf[:], rhs=brow[:],
                                         start=False, stop=True)
                    # e = Exp([-f | s]);  sp = Ln(e_s + 1);  sig = 1/(1+e_f)
                    e_all = wpool.tile([128, 256], F32, tag="e")
                    nc.scalar.activation(e_all[:], ps_e[:], AF.Exp)
                    sp = wpool.tile([128, 128], F32, tag="sp")
                    nc.scalar.activation(sp[:], e_all[:, 128:256], AF.Ln, bias=1.0)
                    den = wpool.tile([128, 128], F32, tag="den")
                    nc.vector.tensor_scalar_add(den[:], e_all[:, 0:128], 1.0)
                    sig = wpool.tile([128, 128], F32, tag="sig")
                    nc.vector.reciprocal(sig[:], den[:])
                    msg = wpool.tile([128, 128], BF16, tag="msg")
                    nc.vector.tensor_mul(msg[:], sig[:], sp[:])
                    S = wpool.tile([128, 128], BF16, tag="S")
                    nc.vector.tensor_tensor(
                        out=S[:],
                        in0=wr_sb[:, t:t + 1].to_broadcast([128, 128]),
                        in1=iota_f[:],
                        op=mybir.AluOpType.is_equal,
                    )
                    # psw[f, n] += msg.T @ S  (segment-sum of the window)
                    nc.tensor.matmul(psw[:], lhsT=msg[:], rhs=S[:],
                                     start=(j == 0), stop=(j == tpw - 1),
                                     skip_group_check=True)
                wsl = slice(w * 128, (w + 1) * 128)
                # copy psum -> agg slice; accum_out = per-feature sum
                nc.scalar.activation(agg_sb[:, wsl], psw[:], AF.Identity,
                                     accum_out=stats_sum[:, w:w + 1])
                # sum of squares via ACT Square + accum_out (the DVE
                # tensor_tensor_reduce path crashes the device on this stack)
                sq = wpool.tile([128, 128], F32, tag="sq")
                nc.scalar.activation(sq[:], psw[:], AF.Square,
                                     accum_out=stats_sq[:, w:w + 1])

            # ---------------- BN statistics + cross-core AllReduce ----------
            st2 = res.tile([128, 2], F32, tag="st2")
            nc.vector.tensor_reduce(st2[:, 0:1], stats_sum[:],
                                    mybir.AxisListType.X, mybir.AluOpType.add)
            nc.vector.tensor_reduce(st2[:, 1:2], stats_sq[:],
                                    mybir.AxisListType.X, mybir.AluOpType.add)
            cin = dram.tile([128, 2], F32)
            cout = dram.tile([128, 2], F32)
            nc.gpsimd.dma_start(cin[:], st2[:])
            nc.gpsimd.collective_compute(
                "AllReduce", mybir.AluOpType.add,
                replica_groups=[list(range(N_CORES))],
                ins=[cin.opt()], outs=[cout.opt()],
            )
            stg = res.tile([128, 2], F32, tag="stg")
            nc.gpsimd.dma_start(stg[:], cout[:])

            mean = res.tile([128, 1], F32, tag="mean")
            nc.vector.tensor_scalar_mul(mean[:], stg[:, 0:1], 1.0 / n_nodes)
            ex2 = res.tile([128, 1], F32, tag="ex2")
            nc.vector.tensor_scalar_mul(ex2[:], stg[:, 1:2], 1.0 / n_nodes)
            m2 = res.tile([128, 1], F32, tag="m2")
            nc.vector.tensor_mul(m2[:], mean[:], mean[:])
            var = res.tile([128, 1], F32, tag="var")
            nc.vector.tensor_sub(var[:], ex2[:], m2[:])
            nc.vector.tensor_scalar_add(var[:], var[:], BN_EPS)
            # rsqrt(v) = Exp(-0.5 * Ln(v)) — keeps everything in one ACT table
            lnv = res.tile([128, 1], F32, tag="lnv")
            nc.scalar.activation(lnv[:], var[:], AF.Ln)
            inv = res.tile([128, 1], F32, tag="inv")
            nc.scalar.activation(inv[:], lnv[:], AF.Exp, scale=-0.5)
            a_col = res.tile([128, 1], F32, tag="acol")
            nc.vector.tensor_mul(a_col[:], inv[:], g_sb[:])
            ma = res.tile([128, 1], F32, tag="ma")
            nc.vector.tensor_mul(ma[:], mean[:], a_col[:])
            b_col = res.tile([128, 1], F32, tag="bcol")
            nc.vector.tensor_sub(b_col[:], b_sb[:], ma[:])

            # ---------------- phase 2: BN apply + softplus -----------------
            for w in range(nw):
                wsl = slice(w * 128, (w + 1) * 128)
                pre = wpool.tile([128, 128], F32, tag="pre")
                nc.scalar.activation(pre[:], agg_sb[:, wsl], AF.Identity,
                                     bias=b_col[:], scale=a_col[:])
                nc.vector.tensor_add(pre[:], pre[:], xsb[:, wsl])
                ex = wpool.tile([128, 128], F32, tag="ex")
                nc.scalar.activation(ex[:], pre[:], AF.Exp)
                ot = wpool.tile([128, 128], F32, tag="ot")
                nc.scalar.activation(ot[:], ex[:], AF.Ln, bias=1.0)
                nc.sync.dma_start(outT[:, wsl], ot[:])

    nc.compile()
    _BUILD_CACHE[key] = nc
    return nc


# --------------------------------------------------------------------------
# entry point
# --------------------------------------------------------------------------

def kernel(x, edge_attr, edge_source, edge_target, Wf, bf, Ws, bs, gamma, beta):
    x = np.asarray(x)
    n_nodes = x.shape[0]
    in_maps, meta = _prepare(x, edge_attr, edge_source, edge_target, Wf, bf, Ws, bs)

    gcol = np.asarray(gamma, dtype=np.float32).reshape(128, 1)
    bcol = np.asarray(beta, dtype=np.float32).reshape(128, 1)
    for m in in_maps:
        m["gcol"] = gcol
        m["bcol"] = bcol

    nc = _build_fused(meta["nw"], meta["tpw"], meta["n_nodes"], meta["with_bias"])
    res = run_bass_kernel_spmd(nc, in_maps, core_ids=list(range(N_CORES)))

    bounds = meta["bounds"]
    out = np.empty((n_nodes, 128), dtype=np.float32)
    for c in range(N_CORES):
        lo, hi = bounds[c], bounds[c + 1]
        out[lo:hi] = np.asarray(res.results[c]["outT"])[:, 0:hi - lo].T
    return out


# revision 11
# speedup vs baseline: 68.4632x; 45.9825x over previous
"""Trainium2 Bass kernel for nn_Node_Convolution (GNN message passing).

Reference computation:
    z   = concat([x[src], x[tgt], edge_attr], -1)           # [E, 384]
    msg = sigmoid(z @ Wf + bf) * softplus(z @ Ws + bs)      # [E, 128]
    agg = segment_sum(msg, src, N)                          # [N, 128]
    out = softplus(x + batchnorm(agg))                      # [N, 128]

Strategy — ONE fused SPMD kernel on 8 NeuronCores (the per-dispatch
overhead of this execution stack is ~85 ms, so a single dispatch wins):
  * Host: sort edges by source node, split nodes into 8 contiguous ranges
    with ~equal edge counts.  Each core's range is cut into 128-node
    windows; each window's edges are packed into a FIXED number (TPW) of
    128-edge tiles (padded), so the instruction stream is identical on
    every core (SPMD) while tile contents differ.
  * Everything on device is feature-major ([feat, node/edge]) so the BN
    statistics live in the partition dim as per-partition scalars:
      - per tile: psum[e, 0:256] = z_tile @ [Wf | Ws] via 3 accumulating
        bf16 matmuls; ACT Sigmoid / ACT Softplus; DVE mul -> msg [e, f]
      - scatter: one-hot S[e, n] = (winrel[e] == n); matmul
        psw[f, n] += msg.T @ S accumulates the window's segment-sum in
        PSUM across its TPW tiles (exact fp32 dedup)
      - window epilogue: ACT copy psw -> agg slice with accum_out giving
        the per-feature sum; DVE tensor_tensor_reduce gives sum-of-squares
  * BN stats ([128, 2] per core) are AllReduce'd across the 8 cores
    INSIDE the kernel (gpsimd collective_compute via DRAM bounce), then
    a = gamma*rsqrt(var+eps), b = beta - mean*a as [128,1] columns.
  * Phase 2 (same dispatch): out[f, n] = ACT Softplus(x + ACT(agg*a + b))
    streamed per window; x is preloaded feature-major.
  * Host: de-transpose the 8 output shards into the full [N, 128] output.

The host does only data layout (sort/shard/gather/transpose); all FLOPs,
the segment-sum, the BN reduction and application run on device.
(SWDGE ucode gather/scatter is non-functional under this stack, so edge
gathers are materialized host-side during sharding.)
"""
import sys
sys.path.insert(0, "/opt/trn_rl_repo")

import numpy as np
import ml_dtypes

from concourse import bass, mybir
import concourse.bacc as bacc
import concourse.tile as tile
from concourse.bass_utils import run_bass_kernel_spmd

F32 = mybir.dt.float32
BF16 = mybir.dt.bfloat16

N_CORES = 8
D = 128
BN_EPS = 1e-5

_BUILD_CACHE = {}


# --------------------------------------------------------------------------
# host-side packing
# --------------------------------------------------------------------------

def _partition_nodes(src_sorted, n_nodes, n_edges):
    """Split nodes into N_CORES contiguous ranges with ~equal edge counts."""
    deg = np.bincount(src_sorted, minlength=n_nodes)
    cum = np.cumsum(deg)
    bounds = [0]
    for k in range(1, N_CORES):
        bounds.append(int(np.searchsorted(cum, k * n_edges / N_CORES)))
    bounds.append(n_nodes)
    return [int(b) for b in bounds], deg


def _prepare(x, edge_attr, edge_source, edge_target, Wf, bf, Ws, bs):
    n_nodes, d = x.shape
    n_edges = edge_source.shape[0]
    src = np.asarray(edge_source).astype(np.int64)
    tgt = np.asarray(edge_target).astype(np.int64)

    order = np.argsort(src, kind="stable")
    src_s = src[order]
    bounds, deg = _partition_nodes(src_s, n_nodes, n_edges)
    cum = np.concatenate([[0], np.cumsum(deg)])

    # uniform kernel structure: nw windows x TPW tiles on every core
    nw = max(-(-(bounds[c + 1] - bounds[c]) // 128) for c in range(N_CORES))
    tpw = 1
    for c in range(N_CORES):
        lo, hi = bounds[c], bounds[c + 1]
        for w in range(-(-(hi - lo) // 128)):
            a = lo + w * 128
            b = min(a + 128, hi)
            tpw = max(tpw, -(-int(cum[b] - cum[a]) // 128))
    n_tiles = nw * tpw
    e_pad = n_tiles * 128
    n_pad = nw * 128

    x32 = np.asarray(x, dtype=np.float32)
    ea32 = np.asarray(edge_attr, dtype=np.float32)
    Wf32 = np.asarray(Wf, dtype=np.float32)
    Ws32 = np.asarray(Ws, dtype=np.float32)
    bf32 = np.asarray(bf, dtype=np.float32)
    bs32 = np.asarray(bs, dtype=np.float32)
    with_bias = bool(np.any(bf32) or np.any(bs32))

    # f-block negated so sigmoid(f) = 1/(1 + Exp(-f)) shares the Exp with
    # softplus (ACT has no Sigmoid/Softplus in the Ln/Exp table)
    wsrc = np.concatenate([-Wf32[0:128], Ws32[0:128]], axis=1).astype(ml_dtypes.bfloat16)
    wtgt = np.concatenate([-Wf32[128:256], Ws32[128:256]], axis=1).astype(ml_dtypes.bfloat16)
    wea = np.concatenate([-Wf32[256:384], Ws32[256:384]], axis=1).astype(ml_dtypes.bfloat16)
    bias_row = np.concatenate([-bf32, bs32]).reshape(1, 256).astype(ml_dtypes.bfloat16)

    x16 = x32.astype(ml_dtypes.bfloat16)
    ea16 = ea32.astype(ml_dtypes.bfloat16)
    in_maps = []
    for c in range(N_CORES):
        lo, hi = bounds[c], bounds[c + 1]
        perm = np.full(e_pad, -1, dtype=np.int64)
        winrel = np.full((128, n_tiles), -1.0, dtype=np.float32)
        for w in range(-(-(hi - lo) // 128)):
            a = lo + w * 128
            b = min(a + 128, hi)
            s0, s1 = int(cum[a]), int(cum[b])
            K = s1 - s0
            if K == 0:
                continue
            base = w * tpw * 128
            perm[base:base + K] = order[s0:s1]
            idx = np.arange(K)
            winrel[idx % 128, w * tpw + idx // 128] = (src_s[s0:s1] - a).astype(np.float32)
        mask = perm >= 0
        pm = perm[mask]

        def _featT(rows):
            a_ = np.zeros((e_pad, D), dtype=ml_dtypes.bfloat16)
            a_[mask] = rows
            return np.ascontiguousarray(a_.T)

        srcT = _featT(x16[src[pm]])
        tgtT = _featT(x16[tgt[pm]])
        eaT = _featT(ea16[pm])
        xT = np.zeros((D, n_pad), dtype=np.float32)
        xT[:, 0:hi - lo] = x32[lo:hi].T
        m = dict(eaT=eaT, srcT=srcT, tgtT=tgtT, winrel=winrel,
                 wsrc=wsrc, wtgt=wtgt, wea=wea, xT=xT)
        if with_bias:
            m["bias_row"] = bias_row
        in_maps.append(m)

    meta = dict(nw=nw, tpw=tpw, n_pad=n_pad, bounds=bounds,
                with_bias=with_bias, n_nodes=n_nodes)
    return in_maps, meta


# --------------------------------------------------------------------------
# fused device kernel
# --------------------------------------------------------------------------

def _build_fused(nw, tpw, n_nodes, with_bias):
    key = ("fused", nw, tpw, n_nodes, with_bias)
    if key in _BUILD_CACHE:
        return _BUILD_CACHE[key]
    n_tiles = nw * tpw
    e_pad = n_tiles * 128
    n_pad = nw * 128
    nc = bacc.Bacc(None, debug=False, num_devices=N_CORES)

    eaT = nc.declare_dram_parameter("eaT", [128, e_pad], BF16, isOutput=False)
    srcT = nc.declare_dram_parameter("srcT", [128, e_pad], BF16, isOutput=False)
    tgtT = nc.declare_dram_parameter("tgtT", [128, e_pad], BF16, isOutput=False)
    winrel = nc.declare_dram_parameter("winrel", [128, n_tiles], F32, isOutput=False)
    wsrc = nc.declare_dram_parameter("wsrc", [128, 256], BF16, isOutput=False)
    wtgt = nc.declare_dram_parameter("wtgt", [128, 256], BF16, isOutput=False)
    wea = nc.declare_dram_parameter("wea", [128, 256], BF16, isOutput=False)
    xT = nc.declare_dram_parameter("xT", [128, n_pad], F32, isOutput=False)
    gcol = nc.declare_dram_parameter("gcol", [128, 1], F32, isOutput=False)
    bcol = nc.declare_dram_parameter("bcol", [128, 1], F32, isOutput=False)
    if with_bias:
        bias_row = nc.declare_dram_parameter("bias_row", [1, 256], BF16, isOutput=False)
    outT = nc.declare_dram_parameter("outT", [128, n_pad], F32, isOutput=True)

    AF = mybir.ActivationFunctionType
    with tile.TileContext(nc) as tc:
        with (
            tc.tile_pool(name="res", bufs=1) as res,
            tc.tile_pool(name="chunk", bufs=2) as cpool,
            tc.tile_pool(name="work", bufs=3) as wpool,
            tc.tile_pool(name="pse", bufs=2, space="PSUM") as pse_pool,
            tc.tile_pool(name="psw", bufs=2, space="PSUM") as psw_pool,
            tc.tile_pool(name="dram", bufs=2, space="DRAM") as dram,
        ):
            # resident constants / state
            w1 = res.tile([128, 256], BF16, tag="w1")
            w2 = res.tile([128, 256], BF16, tag="w2")
            w3 = res.tile([128, 256], BF16, tag="w3")
            nc.sync.dma_start(w1[:], wsrc[:, :])
            nc.sync.dma_start(w2[:], wtgt[:, :])
            nc.sync.dma_start(w3[:], wea[:, :])
            wr_sb = res.tile([128, n_tiles], F32, tag="wr")
            nc.sync.dma_start(wr_sb[:], winrel[:, :])
            xsb = res.tile([128, n_pad], F32, tag="x")
            nc.sync.dma_start(xsb[:], xT[:, :])
            g_sb = res.tile([128, 1], F32, tag="g")
            b_sb = res.tile([128, 1], F32, tag="b")
            nc.sync.dma_start(g_sb[:], gcol[:, :])
            nc.sync.dma_start(b_sb[:], bcol[:, :])
            iota_i = res.tile([128, 128], mybir.dt.int32, tag="ii")
            nc.gpsimd.iota(iota_i[:], pattern=[[1, 128]], base=0, channel_multiplier=0)
            iota_f = res.tile([128, 128], F32, tag="if")
            nc.vector.tensor_copy(iota_f[:], iota_i[:])
            if with_bias:
                ones_bf = res.tile([1, 128], BF16, tag="ob")
                nc.vector.memset(ones_bf[:], 1.0)
                brow = res.tile([1, 256], BF16, tag="br")
                nc.sync.dma_start(brow[:], bias_row[:, :])

            agg_sb = res.tile([128, n_pad], F32, tag="agg")
            stats_sum = res.tile([128, nw], F32, tag="ssum")
            stats_sq = res.tile([128, nw], F32, tag="ssq")

            # ---------------- phase 1: edge messages + segment sum ----------
            for w in range(nw):
                c0 = w * tpw * 128
                ea_c = cpool.tile([128, tpw * 128], BF16, tag="ea")
                src_c = cpool.tile([128, tpw * 128], BF16, tag="src")
                tgt_c = cpool.tile([128, tpw * 128], BF16, tag="tgt")
                nc.sync.dma_start(ea_c[:], eaT[:, c0:c0 + tpw * 128])
                nc.sync.dma_start(src_c[:], srcT[:, c0:c0 + tpw * 128])
                nc.sync.dma_start(tgt_c[:], tgtT[:, c0:c0 + tpw * 128])
                psw = psw_pool.tile([128, 128], F32, tag="psw")
                for j in range(tpw):
                    t = w * tpw + j
                    sl = slice(j * 128, (j + 1) * 128)
                    ps_e = pse_pool.tile([128, 256], F32, tag="pse")
                    nc.tensor.matmul(ps_e[:], lhsT=src_c[:, sl], rhs=w1[:],
                                     start=True, stop=False)
                    nc.tensor.matmul(ps_e[:], lhsT=tgt_c[:, sl], rhs=w2[:],
                                     start=False, stop=False)
                    nc.tensor.matmul(ps_e[:], lhsT=ea_c[:, sl], rhs=w3[:],
                                     start=False, stop=not with_bias)
                    if with_bias:
                        nc.tensor.matmul(ps_e[:], lhsT=ones_bf[:], rhs=brow[:],
                                         start=False, stop=True)
                    # e = Exp([-f | s]);  sp = Ln(e_s + 1);  sig = 1/(1+e_f)
                    e_all = wpool.tile([128, 256], F32, tag="e")
                    nc.scalar.activation(e_all[:], ps_e[:], AF.Exp)
                    sp = wpool.tile([128, 128], F32, tag="sp")
                    nc.scalar.activation(sp[:], e_all[:, 128:256], AF.Ln, bias=1.0)
                    den = wpool.tile([128, 128], F32, tag="den")
                    nc.vector.tensor_scalar_add(den[:], e_all[:, 0:128], 1.0)
                    sig = wpool.tile([128, 128], F32, tag="sig")
                    nc.vector.reciprocal(sig[:], den[:])
                    msg = wpool.tile([128, 128], BF16, tag="msg")
                    nc.vector.tensor_mul(msg[:], sig[:], sp[:])
                    S = wpool.tile([128, 128], BF16, tag="S")
                    nc.vector.tensor_tensor(
                        out=S[:],
                        in0=wr_sb[:, t:t + 1].to_broadcast([128, 128]),
                        in1=iota_f[:],
                        op=mybir.AluOpType.is_equal,
                    )
                    # psw[f, n] += msg.T @ S  (segment-sum of the window)
                    nc.tensor.matmul(psw[:], lhsT=msg[:], rhs=S[:],
                                     start=(j == 0), stop=(j == tpw - 1),
                                     skip_group_check=True)
                wsl = slice(w * 128, (w + 1) * 128)
                # copy psum -> agg slice; accum_out = per-feature sum
                nc.scalar.activation(agg_sb[:, wsl], psw[:], AF.Identity,
                                     accum_out=stats_sum[:, w:w + 1])
                # sum of squares via ACT Square + accum_out (the DVE
                # tensor_tensor_reduce path crashes the device on this stack)
                sq = wpool.tile([128, 128], F32, tag="sq")
                nc.scalar.activation(sq[:], psw[:], AF.Square,
                                     accum_out=stats_sq[:, w:w + 1])

            # ---------------- BN statistics + cross-core AllReduce ----------
            st2 = res.tile([128, 2], F32, tag="st2")
            nc.vector.tensor_reduce(st2[:, 0:1], stats_sum[:],
                                    mybir.AxisListType.X, mybir.AluOpType.add)
            nc.vector.tensor_reduce(st2[:, 1:2], stats_sq[:],
                                    mybir.AxisListType.X, mybir.AluOpType.add)
            cin = dram.tile([128, 2], F32)
            cout = dram.tile([128, 2], F32)
            nc.gpsimd.dma_start(cin[:], st2[:])
            nc.gpsimd.collective_compute(
                "AllReduce", mybir.AluOpType.add,
                replica_groups=[list(range(N_CORES))],
                ins=[cin.opt()], outs=[cout.opt()],
            )
            stg = res.tile([128, 2], F32, tag="stg")
            nc.gpsimd.dma_start(stg[:], cout[:])

            mean = res.tile([128, 1], F32, tag="mean")
            nc.vector.tensor_scalar_mul(mean[:], stg[:, 0:1], 1.0 / n_nodes)
            ex2 = res.tile([128, 1], F32, tag="ex2")
            nc.vector.tensor_scalar_mul(ex2[:], stg[:, 1:2], 1.0 / n_nodes)
            m2 = res.tile([128, 1], F32, tag="m2")
            nc.vector.tensor_mul(m2[:], mean[:], mean[:])
            var = res.tile([128, 1], F32, tag="var")
            nc.vector.tensor_sub(var[:], ex2[:], m2[:])
            nc.vector.tensor_scalar_add(var[:], var[:], BN_EPS)
            # rsqrt(v) = Exp(-0.5 * Ln(v)) — keeps everything in one ACT table
            lnv = res.tile([128, 1], F32, tag="lnv")
            nc.scalar.activation(lnv[:], var[:], AF.Ln)
            inv = res.tile([128, 1], F32, tag="inv")
            nc.scalar.activation(inv[:], lnv[:], AF.Exp, scale=-0.5)
            a_col = res.tile([128, 1], F32, tag="acol")
            nc.vector.tensor_mul(a_col[:], inv[:], g_sb[:])
            ma = res.tile([128, 1], F32, tag="ma")
            nc.vector.tensor_mul(ma[:], mean[:], a_col[:])
            b_col = res.tile([128, 1], F32, tag="bcol")
            nc.vector.tensor_sub(b_col[:], b_sb[:], ma[:])

            # ---------------- phase 2: BN apply + softplus -----------------
            for w in range(nw):
                wsl = slice(w * 128, (w + 1) * 128)
                pre = wpool.tile([128, 128], F32, tag="pre")
                nc.scalar.activation(pre[:], agg_sb[:, wsl], AF.Identity,
                                     bias=b_col[:], scale=a_col[:])
                nc.vector.tensor_add(pre[:], pre[:], xsb[:, wsl])
                ex = wpool.tile([128, 128], F32, tag="ex")
                nc.scalar.activation(ex[:], pre[:], AF.Exp)
                ot = wpool.tile([128, 128], F32, tag="ot")
                nc.scalar.activation(ot[:], ex[:], AF.Ln, bias=1.0)
                nc.sync.dma_start(outT[:, wsl], ot[:])

    nc.compile()
    _BUILD_CACHE[key] = nc
    return nc


# --------------------------------------------------------------------------
# entry point
# --------------------------------------------------------------------------

def kernel(x, edge_attr, edge_source, edge_target, Wf, bf, Ws, bs, gamma, beta):
    x = np.asarray(x)
    n_nodes = x.shape[0]
    in_maps, meta = _prepare(x, edge_attr, edge_source, edge_target, Wf, bf, Ws, bs)

    gcol = np.asarray(gamma, dtype=np.float32).reshape(128, 1)
    bcol = np.asarray(beta, dtype=np.float32).reshape(128, 1)
    for m in in_maps:
        m["gcol"] = gcol
        m["bcol"] = bcol

    nc = _build_fused(meta["nw"], meta["tpw"], meta["n_nodes"], meta["with_bias"])
    res = run_bass_kernel_spmd(nc, in_maps, core_ids=list(range(N_CORES)))

    bounds = meta["bounds"]
    out = np.empty((n_nodes, 128), dtype=np.float32)
    for c in range(N_CORES):
        lo, hi = bounds[c], bounds[c + 1]
        out[lo:hi] = np.asarray(res.results[c]["outT"])[:, 0:hi - lo].T
    return out


# revision 18
# speedup vs baseline: 75.3458x; 1.1005x over previous
"""Trainium2 Bass kernel for nn_Node_Convolution (GNN message passing).

Reference computation:
    z   = concat([x[src], x[tgt], edge_attr], -1)           # [E, 384]
    msg = sigmoid(z @ Wf + bf) * softplus(z @ Ws + bs)      # [E, 128]
    agg = segment_sum(msg, src, N)                          # [N, 128]
    out = softplus(x + batchnorm(agg))                      # [N, 128]

Strategy — ONE fused SPMD kernel on 8 NeuronCores (the per-dispatch
overhead of this execution stack is ~85 ms, so a single dispatch wins):
  * Host: sort edges by source node, split nodes into 8 contiguous ranges
    with ~equal edge counts.  Each core's range is cut into 128-node
    windows; each window's edges are packed into a FIXED number (TPW) of
    128-edge tiles (padded), so the instruction stream is identical on
    every core (SPMD) while tile contents differ.
  * Everything on device is feature-major ([feat, node/edge]) so the BN
    statistics live in the partition dim as per-partition scalars:
      - per tile: psum[e, 0:256] = z_tile @ [Wf | Ws] via 3 accumulating
        bf16 matmuls; ACT Sigmoid / ACT Softplus; DVE mul -> msg [e, f]
      - scatter: one-hot S[e, n] = (winrel[e] == n); matmul
        psw[f, n] += msg.T @ S accumulates the window's segment-sum in
        PSUM across its TPW tiles (exact fp32 dedup)
      - window epilogue: ACT copy psw -> agg slice with accum_out giving
        the per-feature sum; DVE tensor_tensor_reduce gives sum-of-squares
  * BN stats ([128, 2] per core) are AllReduce'd across the 8 cores
    INSIDE the kernel (gpsimd collective_compute via DRAM bounce), then
    a = gamma*rsqrt(var+eps), b = beta - mean*a as [128,1] columns.
  * Phase 2 (same dispatch): out[f, n] = ACT Softplus(x + ACT(agg*a + b))
    streamed per window; x is preloaded feature-major.
  * Host: de-transpose the 8 output shards into the full [N, 128] output.

The host does only data layout (sort/shard/gather/transpose); all FLOPs,
the segment-sum, the BN reduction and application run on device.
(SWDGE ucode gather/scatter is non-functional under this stack, so edge
gathers are materialized host-side during sharding.)
"""
import sys
sys.path.insert(0, "/opt/trn_rl_repo")

import numpy as np
import ml_dtypes

from concourse import bass, mybir
import concourse.bacc as bacc
import concourse.tile as tile
from concourse.bass_utils import run_bass_kernel_spmd

F32 = mybir.dt.float32
BF16 = mybir.dt.bfloat16

N_CORES = 8
D = 128
BN_EPS = 1e-5

_BUILD_CACHE = {}


# --------------------------------------------------------------------------
# host-side packing
# --------------------------------------------------------------------------

def _partition_nodes(src_sorted, n_nodes, n_edges):
    """Split nodes into N_CORES contiguous ranges with ~equal edge counts."""
    deg = np.bincount(src_sorted, minlength=n_nodes)
    cum = np.cumsum(deg)
    bounds = [0]
    for k in range(1, N_CORES):
        bounds.append(int(np.searchsorted(cum, k * n_edges / N_CORES)))
    bounds.append(n_nodes)
    return [int(b) for b in bounds], deg


def _prepare(x, edge_attr, edge_source, edge_target, Wf, bf, Ws, bs):
    n_nodes, d = x.shape
    n_edges = edge_source.shape[0]
    src = np.asarray(edge_source).astype(np.int64)
    tgt = np.asarray(edge_target).astype(np.int64)

    order = np.argsort(src, kind="stable")
    src_s = src[order]
    bounds, deg = _partition_nodes(src_s, n_nodes, n_edges)
    cum = np.concatenate([[0], np.cumsum(deg)])

    # uniform kernel structure: nw windows x TPW tiles on every core
    nw = max(-(-(bounds[c + 1] - bounds[c]) // 128) for c in range(N_CORES))
    tpw = 1
    for c in range(N_CORES):
        lo, hi = bounds[c], bounds[c + 1]
        for w in range(-(-(hi - lo) // 128)):
            a = lo + w * 128
            b = min(a + 128, hi)
            tpw = max(tpw, -(-int(cum[b] - cum[a]) // 128))
    n_tiles = nw * tpw
    e_pad = n_tiles * 128
    n_pad = nw * 128

    x32 = np.asarray(x, dtype=np.float32)
    ea32 = np.asarray(edge_attr, dtype=np.float32)
    Wf32 = np.asarray(Wf, dtype=np.float32)
    Ws32 = np.asarray(Ws, dtype=np.float32)
    bf32 = np.asarray(bf, dtype=np.float32)
    bs32 = np.asarray(bs, dtype=np.float32)
    with_bias = bool(np.any(bf32) or np.any(bs32))

    # f-block negated so sigmoid(f) = 1/(1 + Exp(-f)) shares the Exp with
    # softplus (ACT has no Sigmoid/Softplus in the Ln/Exp table)
    wsrc = np.concatenate([-Wf32[0:128], Ws32[0:128]], axis=1).astype(ml_dtypes.bfloat16)
    wtgt = np.concatenate([-Wf32[128:256], Ws32[128:256]], axis=1).astype(ml_dtypes.bfloat16)
    wea = np.concatenate([-Wf32[256:384], Ws32[256:384]], axis=1).astype(ml_dtypes.bfloat16)
    bias_row = np.concatenate([-bf32, bs32]).reshape(1, 256).astype(ml_dtypes.bfloat16)

    x16 = x32.astype(ml_dtypes.bfloat16)
    ea16 = ea32.astype(ml_dtypes.bfloat16)
    in_maps = []
    for c in range(N_CORES):
        lo, hi = bounds[c], bounds[c + 1]
        perm = np.full(e_pad, -1, dtype=np.int64)
        winrel = np.full((128, n_tiles), -1.0, dtype=np.float32)
        for w in range(-(-(hi - lo) // 128)):
            a = lo + w * 128
            b = min(a + 128, hi)
            s0, s1 = int(cum[a]), int(cum[b])
            K = s1 - s0
            if K == 0:
                continue
            base = w * tpw * 128
            perm[base:base + K] = order[s0:s1]
            idx = np.arange(K)
            winrel[idx % 128, w * tpw + idx // 128] = (src_s[s0:s1] - a).astype(np.float32)
        mask = perm >= 0
        pm = perm[mask]

        def _featT(rows):
            a_ = np.zeros((e_pad, D), dtype=ml_dtypes.bfloat16)
            a_[mask] = rows
            return np.ascontiguousarray(a_.T)

        srcT = _featT(x16[src[pm]])
        tgtT = _featT(x16[tgt[pm]])
        eaT = _featT(ea16[pm])
        xT = np.zeros((D, n_pad), dtype=np.float32)
        xT[:, 0:hi - lo] = x32[lo:hi].T
        m = dict(eaT=eaT, srcT=srcT, tgtT=tgtT, winrel=winrel,
                 wsrc=wsrc, wtgt=wtgt, wea=wea, xT=xT)
        if with_bias:
            m["bias_row"] = bias_row
        in_maps.append(m)

    meta = dict(nw=nw, tpw=tpw, n_pad=n_pad, bounds=bounds,
                with_bias=with_bias, n_nodes=n_nodes)
    return in_maps, meta


# --------------------------------------------------------------------------
# fused device kernel
# --------------------------------------------------------------------------

def _build_fused(nw, tpw, n_nodes, with_bias):
    key = ("fused", nw, tpw, n_nodes, with_bias)
    if key in _BUILD_CACHE:
        return _BUILD_CACHE[key]
    n_tiles = nw * tpw
    e_pad = n_tiles * 128
    n_pad = nw * 128
    nc = bacc.Bacc(None, debug=False, num_devices=N_CORES)

    eaT = nc.declare_dram_parameter("eaT", [128, e_pad], BF16, isOutput=False)
    srcT = nc.declare_dram_parameter("srcT", [128, e_pad], BF16, isOutput=False)
    tgtT = nc.declare_dram_parameter("tgtT", [128, e_pad], BF16, isOutput=False)
    winrel = nc.declare_dram_parameter("winrel", [128, n_tiles], F32, isOutput=False)
    wsrc = nc.declare_dram_parameter("wsrc", [128, 256], BF16, isOutput=False)
    wtgt = nc.declare_dram_parameter("wtgt", [128, 256], BF16, isOutput=False)
    wea = nc.declare_dram_parameter("wea", [128, 256], BF16, isOutput=False)
    xT = nc.declare_dram_parameter("xT", [128, n_pad], F32, isOutput=False)
    gcol = nc.declare_dram_parameter("gcol", [128, 1], F32, isOutput=False)
    bcol = nc.declare_dram_parameter("bcol", [128, 1], F32, isOutput=False)
    if with_bias:
        bias_row = nc.declare_dram_parameter("bias_row", [1, 256], BF16, isOutput=False)
    outT = nc.declare_dram_parameter("outT", [128, n_pad], F32, isOutput=True)

    AF = mybir.ActivationFunctionType
    with tile.TileContext(nc) as tc:
        with (
            tc.tile_pool(name="res", bufs=1) as res,
            tc.tile_pool(name="chunk", bufs=4) as cpool,
            tc.tile_pool(name="work", bufs=3) as wpool,
            tc.tile_pool(name="epool", bufs=2 * tpw) as epool,
            tc.tile_pool(name="sppool", bufs=2 * tpw) as sppool,
            tc.tile_pool(name="p2pre", bufs=10) as p2pre,
            tc.tile_pool(name="p2ex", bufs=10) as p2ex,
            tc.tile_pool(name="p2ot", bufs=10) as p2ot,
            tc.tile_pool(name="pse", bufs=3, space="PSUM") as pse_pool,
            tc.tile_pool(name="psw", bufs=2, space="PSUM") as psw_pool,
            tc.tile_pool(name="dram", bufs=2, space="DRAM") as dram,
        ):
            # resident constants / state
            w1 = res.tile([128, 256], BF16, tag="w1")
            w2 = res.tile([128, 256], BF16, tag="w2")
            w3 = res.tile([128, 256], BF16, tag="w3")
            nc.sync.dma_start(w1[:], wsrc[:, :])
            nc.sync.dma_start(w2[:], wtgt[:, :])
            nc.sync.dma_start(w3[:], wea[:, :])
            wr_sb = res.tile([128, n_tiles], F32, tag="wr")
            nc.sync.dma_start(wr_sb[:], winrel[:, :])
            xsb = res.tile([128, n_pad], F32, tag="x")
            nc.sync.dma_start(xsb[:], xT[:, :])
            g_sb = res.tile([128, 1], F32, tag="g")
            b_sb = res.tile([128, 1], F32, tag="b")
            nc.sync.dma_start(g_sb[:], gcol[:, :])
            nc.sync.dma_start(b_sb[:], bcol[:, :])
            iota_i = res.tile([128, 128], mybir.dt.int32, tag="ii")
            nc.gpsimd.iota(iota_i[:], pattern=[[1, 128]], base=0, channel_multiplier=0)
            iota_f = res.tile([128, 128], F32, tag="if")
            nc.vector.tensor_copy(iota_f[:], iota_i[:])
            if with_bias:
                ones_bf = res.tile([1, 128], BF16, tag="ob")
                nc.vector.memset(ones_bf[:], 1.0)
                brow = res.tile([1, 256], BF16, tag="br")
                nc.sync.dma_start(brow[:], bias_row[:, :])

            agg_sb = res.tile([128, n_pad], F32, tag="agg")
            stats_sum = res.tile([128, nw], F32, tag="ssum")
            stats_sq = res.tile([128, nw], F32, tag="ssq")

            # ---------------- phase 1: edge messages + segment sum ----------
            # staged in groups of WG windows: all matmul+Exp first, then all
            # Lns, then the DVE/scatter chain. Keeps the ACT queue's Exp/Ln
            # runs contiguous: Exp and Ln live in different ACT function
            # tables, and interleaving them costs a ~1.3us table reload per
            # switch (the profile showed 664 reloads = 62% of kernel time
            # when naively interleaved).
            WG = 2
            for wg0 in range(0, nw, WG):
                wgs = range(wg0, min(wg0 + WG, nw))
                chunks = {}
                for w in wgs:
                    c0 = w * tpw * 128
                    ea_c = cpool.tile([128, tpw * 128], BF16, tag="ea")
                    src_c = cpool.tile([128, tpw * 128], BF16, tag="src")
                    tgt_c = cpool.tile([128, tpw * 128], BF16, tag="tgt")
                    nc.sync.dma_start(ea_c[:], eaT[:, c0:c0 + tpw * 128])
                    nc.sync.dma_start(src_c[:], srcT[:, c0:c0 + tpw * 128])
                    nc.sync.dma_start(tgt_c[:], tgtT[:, c0:c0 + tpw * 128])
                    chunks[w] = (ea_c, src_c, tgt_c)
                es = {}
                for w in wgs:
                    ea_c, src_c, tgt_c = chunks[w]
                    for j in range(tpw):
                        sl = slice(j * 128, (j + 1) * 128)
                        ps_e = pse_pool.tile([128, 256], F32, tag="pse")
                        nc.tensor.matmul(ps_e[:], lhsT=src_c[:, sl], rhs=w1[:],
                                         start=True, stop=False)
                        nc.tensor.matmul(ps_e[:], lhsT=tgt_c[:, sl], rhs=w2[:],
                                         start=False, stop=False)
                        nc.tensor.matmul(ps_e[:], lhsT=ea_c[:, sl], rhs=w3[:],
                                         start=False, stop=not with_bias)
                        if with_bias:
                            nc.tensor.matmul(ps_e[:], lhsT=ones_bf[:], rhs=brow[:],
                                             start=False, stop=True)
                        # e = Exp([-f | s])
                        e_all = epool.tile([128, 256], F32, tag="e")
                        nc.scalar.activation(e_all[:], ps_e[:], AF.Exp)
                        es[(w, j)] = e_all
                sps = {}
                for w in wgs:
                    for j in range(tpw):
                        # sp = Ln(e_s + 1)   (softplus)
                        sp = sppool.tile([128, 128], F32, tag="sp")
                        nc.scalar.activation(sp[:], es[(w, j)][:, 128:256],
                                             AF.Ln, bias=1.0)
                        sps[(w, j)] = sp
                for w in wgs:
                    psw = psw_pool.tile([128, 128], F32, tag="psw")
                    for j in range(tpw):
                        t = w * tpw + j
                        # sig = 1/(1+e_f);  msg = sig * sp
                        den = wpool.tile([128, 128], F32, tag="den")
                        nc.vector.tensor_scalar_add(den[:], es[(w, j)][:, 0:128], 1.0)
                        sig = wpool.tile([128, 128], F32, tag="sig")
                        nc.vector.reciprocal(sig[:], den[:])
                        msg = wpool.tile([128, 128], BF16, tag="msg")
                        nc.vector.tensor_mul(msg[:], sig[:], sps[(w, j)][:])
                        S = wpool.tile([128, 128], BF16, tag="S")
                        nc.vector.tensor_tensor(
                            out=S[:],
                            in0=wr_sb[:, t:t + 1].to_broadcast([128, 128]),
                            in1=iota_f[:],
                            op=mybir.AluOpType.is_equal,
                        )
                        # psw[f, n] += msg.T @ S  (segment-sum of the window)
                        nc.tensor.matmul(psw[:], lhsT=msg[:], rhs=S[:],
                                         start=(j == 0), stop=(j == tpw - 1),
                                         skip_group_check=True)
                    wsl = slice(w * 128, (w + 1) * 128)
                    # copy psum -> agg slice; accum_out = per-feature sum
                    nc.scalar.activation(agg_sb[:, wsl], psw[:], AF.Identity,
                                         accum_out=stats_sum[:, w:w + 1])
                    # sum of squares via ACT Square + accum_out (the DVE
                    # tensor_tensor_reduce path crashes the device here)
                    sq = wpool.tile([128, 128], F32, tag="sq")
                    nc.scalar.activation(sq[:], psw[:], AF.Square,
                                         accum_out=stats_sq[:, w:w + 1])

            # ---------------- BN statistics + cross-core AllReduce ----------
            st2 = res.tile([128, 2], F32, tag="st2")
            nc.vector.tensor_reduce(st2[:, 0:1], stats_sum[:],
                                    mybir.AxisListType.X, mybir.AluOpType.add)
            nc.vector.tensor_reduce(st2[:, 1:2], stats_sq[:],
                                    mybir.AxisListType.X, mybir.AluOpType.add)
            cin = dram.tile([128, 2], F32)
            cout = dram.tile([128, 2], F32)
            nc.gpsimd.dma_start(cin[:], st2[:])
            nc.gpsimd.collective_compute(
                "AllReduce", mybir.AluOpType.add,
                replica_groups=[list(range(N_CORES))],
                ins=[cin.opt()], outs=[cout.opt()],
            )
            stg = res.tile([128, 2], F32, tag="stg")
            nc.gpsimd.dma_start(stg[:], cout[:])

            mean = res.tile([128, 1], F32, tag="mean")
            nc.vector.tensor_scalar_mul(mean[:], stg[:, 0:1], 1.0 / n_nodes)
            ex2 = res.tile([128, 1], F32, tag="ex2")
            nc.vector.tensor_scalar_mul(ex2[:], stg[:, 1:2], 1.0 / n_nodes)
            m2 = res.tile([128, 1], F32, tag="m2")
            nc.vector.tensor_mul(m2[:], mean[:], mean[:])
            var = res.tile([128, 1], F32, tag="var")
            nc.vector.tensor_sub(var[:], ex2[:], m2[:])
            nc.vector.tensor_scalar_add(var[:], var[:], BN_EPS)
            # rsqrt(v) = Exp(-0.5 * Ln(v)) — keeps everything in one ACT table
            lnv = res.tile([128, 1], F32, tag="lnv")
            nc.scalar.activation(lnv[:], var[:], AF.Ln)
            inv = res.tile([128, 1], F32, tag="inv")
            nc.scalar.activation(inv[:], lnv[:], AF.Exp, scale=-0.5)
            a_col = res.tile([128, 1], F32, tag="acol")
            nc.vector.tensor_mul(a_col[:], inv[:], g_sb[:])
            ma = res.tile([128, 1], F32, tag="ma")
            nc.vector.tensor_mul(ma[:], mean[:], a_col[:])
            b_col = res.tile([128, 1], F32, tag="bcol")
            nc.vector.tensor_sub(b_col[:], b_sb[:], ma[:])

            # ---------------- phase 2: BN apply + softplus -----------------
            # batched in groups so the ACT queue sees runs of the same
            # function (Identity*G, Exp*G, Ln*G) -> 2 table loads per group
            G = 10
            for w0 in range(0, nw, G):
                ws = range(w0, min(w0 + G, nw))
                pres, exs = [], []
                for w in ws:
                    wsl = slice(w * 128, (w + 1) * 128)
                    pre = p2pre.tile([128, 128], F32, tag="pre")
                    nc.scalar.activation(pre[:], agg_sb[:, wsl], AF.Identity,
                                         bias=b_col[:], scale=a_col[:])
                    nc.vector.tensor_add(pre[:], pre[:], xsb[:, wsl])
                    pres.append(pre)
                for i, w in enumerate(ws):
                    ex = p2ex.tile([128, 128], F32, tag="ex")
                    nc.scalar.activation(ex[:], pres[i][:], AF.Exp)
                    exs.append(ex)
                for i, w in enumerate(ws):
                    wsl = slice(w * 128, (w + 1) * 128)
                    ot = p2ot.tile([128, 128], F32, tag="ot")
                    nc.scalar.activation(ot[:], exs[i][:], AF.Ln, bias=1.0)
                    nc.sync.dma_start(outT[:, wsl], ot[:])

    nc.compile()
    _BUILD_CACHE[key] = nc
    return nc


# --------------------------------------------------------------------------
# entry point
# --------------------------------------------------------------------------

def kernel(x, edge_attr, edge_source, edge_target, Wf, bf, Ws, bs, gamma, beta):
    x = np.asarray(x)
    n_nodes = x.shape[0]
    in_maps, meta = _prepare(x, edge_attr, edge_source, edge_target, Wf, bf, Ws, bs)

    gcol = np.asarray(gamma, dtype=np.float32).reshape(128, 1)
    bcol = np.asarray(beta, dtype=np.float32).reshape(128, 1)
    for m in in_maps:
        m["gcol"] = gcol
        m["bcol"] = bcol

    nc = _build_fused(meta["nw"], meta["tpw"], meta["n_nodes"], meta["with_bias"])
    res = run_bass_kernel_spmd(nc, in_maps, core_ids=list(range(N_CORES)))

    bounds = meta["bounds"]
    out = np.empty((n_nodes, 128), dtype=np.float32)
    for c in range(N_CORES):
        lo, hi = bounds[c], bounds[c + 1]
        out[lo:hi] = np.asarray(res.results[c]["outT"])[:, 0:hi - lo].T
    return out
